# revision 19
# baseline (speedup 1.0000x reference)
"""BEV feature extractor (scatter-max -> 1x1 conv -> BN(train) -> ReLU) on 8 TRN2 cores.

Sharding: data-parallel over (batch, y-strip) -> 8 shards, BN stats all-reduced.

v1 design (fp16 data paths, ~3x less HBM traffic + 4x PE rate vs the f32 baseline):

  1. Host packs each shard: occupied cells of SLOT_BLKS consecutive 128-cell
     blocks form one 128-row *slot*; root (first) points go into the per-region
     r0 tensors (fp16). Colliding extra points go into fold batches of 128
     (exf), with the root rows duplicated alongside (fi) so no device gather is
     needed. A per-batch level schedule bounds collision depth.
  2. Device folds: f = max(fi, exf levels) on DVE, then indirect-scatters f
     back into r0 *in place* (region-split r0 keeps the 4 fold chains
     independent). V tiles [128, slots, C+1] (fp16, fused ones column) load
     straight from the folded r0 -- no DRAM->DRAM comb copy.
  3. PE accumulates sig = sum_s V_s^T [V_s | 1] (fp16 in, f32 PSUM), projects
     the per-core moments q_o = w_o^T Sigma w_o, m_o = w_o . sv locally, and a
     tiny [128, 2*OCH] AllReduce(+) produces global BN stats:
     mean = m/N, var = q/N - mean^2, a = gamma/sqrt(var+eps), b = beta-mean*a.
  4. Phase C per slot-pair: the 0/1 selection matrix is rebuilt on device from
     a small row-index tensor (selrow) via a K=32 broadcast matmul + DVE
     is_equal (kills the 20MB/core sel load of the baseline); GT = V_s^T @ Sel
     densifies+transposes; conv = W^T_chunk @ GT; ACT applies relu(x*a+b) and
     the result streams out as fp16 (halves the output write).
     The first PBN pairs buffer their conv output in SBUF (fp16) so PE/DVE run
     through the AllReduce window; their ACT+store is emitted after the BN
     constants so only the scalar engine waits on the collective.
"""

import math
from dataclasses import dataclass

import numpy as np

import concourse.bass as bass
import concourse.tile as tile
from concourse import bacc, mybir
from concourse.bass_utils import run_bass_kernel_spmd

F32 = mybir.dt.float32
F16 = mybir.dt.float16
I32 = mybir.dt.int32


@dataclass(frozen=True)
class Geo:
    B: int = 2
    H: int = 400
    W: int = 400
    C: int = 128            # input channels (= partition count)
    O: int = 256            # output channels (multiple of 128)
    NSTRIP: int = 4         # y-strips per batch; B*NSTRIP = 8 cores
    SLOT_BLKS: int = 2      # 128-cell blocks packed per 128-row slot
    NB: int = 6             # fold batches per region (128 roots each)
    NREG: int = 4           # slot regions (independent r0 tensors)
    LVLS: tuple = (5, 2)    # per-batch fold depth; batches beyond get depth 1
    PBN: int = 56           # pairs whose conv output is SBUF-buffered pre-BN
    EPS: float = 1e-5

    @property
    def ystrip(self):
        return self.H // self.NSTRIP

    @property
    def cells(self):
        return self.ystrip * self.W

    @property
    def ncores(self):
        return self.B * self.NSTRIP

    @property
    def slot_cells(self):
        return 128 * self.SLOT_BLKS

    @property
    def nslot(self):
        return math.ceil(self.cells / self.slot_cells)

    @property
    def npairs(self):
        return math.ceil(self.nslot / 2)

    @property
    def lvls(self):
        return tuple(self.LVLS) + (1,) * (self.NB - len(self.LVLS))

    @property
    def npair(self):                 # (batch, level) pairs
        return sum(self.lvls)

    @property
    def ncell_total(self):
        return self.B * self.H * self.W

    @property
    def rs(self):                    # slots per region
        return math.ceil(self.nslot / self.NREG)

    @property
    def reg_bounds(self):
        out = []
        for reg in range(self.NREG):
            lo = min(reg * self.rs, self.nslot)
            hi = self.nslot if reg == self.NREG - 1 else min(
                (reg + 1) * self.rs, self.nslot)
            out.append((lo, hi))
        return out


GEO = Geo()


# --------------------------------------------------------------------------
# host-side shard prep
# --------------------------------------------------------------------------

def prep_shard(g: Geo, feats: np.ndarray, cell: np.ndarray) -> dict:
    """feats [n, C] fp16, cell [n] int in [0, g.cells)."""
    C = g.C
    order = np.argsort(cell, kind="stable")
    cell_s = cell[order]
    feats_s = feats[order]
    uniq, seg_start, inverse, counts = np.unique(
        cell_s, return_index=True, return_inverse=True, return_counts=True
    )
    rank = np.arange(len(cell_s)) - seg_start[inverse]

    # --- slot packing: cell j -> slot j // slot_cells; occupied cells of a
    # slot occupy consecutive rows (cell order) within the slot's 128 rows.
    slot_of_uniq = uniq // g.slot_cells
    occ_per_slot = np.zeros(g.nslot, np.int64)
    np.add.at(occ_per_slot, slot_of_uniq, 1)
    assert occ_per_slot.max(initial=0) <= 128, (
        f"slot overflow: {occ_per_slot.max()}"
    )
    first_of_slot = np.zeros(g.nslot, np.int64)
    first_of_slot[1:] = np.cumsum(occ_per_slot)[:-1]
    row_in_slot = np.arange(len(uniq)) - first_of_slot[slot_of_uniq]
    rowid = slot_of_uniq * 128 + row_in_slot          # global packed row

    lvls = g.lvls
    nbr = len(lvls)
    pair_base = np.cumsum((0,) + lvls[:-1])
    exi = np.zeros((128, nbr * g.NREG), np.int32)
    exf = np.zeros((128, g.npair * g.NREG, C), np.float16)
    pos_in_me = np.zeros(len(uniq), np.int64)
    batch_of = np.zeros(len(uniq), np.int64)
    reg_lo = np.zeros(len(uniq), np.int64)
    shard = {}
    for reg, (lo_s, hi_s) in enumerate(g.reg_bounds):
        ns_r = hi_s - lo_s
        exi[:, reg * nbr : (reg + 1) * nbr] = (
            np.arange(128)[:, None] * (ns_r + 1) + ns_r   # per-lane dump slot
        )
        inreg = (counts > 1) & (slot_of_uniq >= lo_s) & (slot_of_uniq < hi_s)
        ord_me = np.argsort(-counts[inreg], kind="stable")
        me_uniq = np.flatnonzero(inreg)[ord_me]
        nme = len(me_uniq)
        assert nme <= 128 * nbr, f"region fold capacity exceeded: {nme}"
        bi = np.arange(nme) // 128
        pi = np.arange(nme) % 128
        assert (counts[me_uniq] - 1 <= np.asarray(lvls)[bi]).all(), (
            "collision depth exceeds fold schedule"
        )
        exi[pi, reg * nbr + bi] = (
            row_in_slot[me_uniq] * (ns_r + 1) + (slot_of_uniq[me_uniq] - lo_s)
        ).astype(np.int32)
        pos_in_me[me_uniq] = np.arange(nme)
        batch_of[me_uniq] = reg * nbr + bi
        reg_lo[me_uniq] = lo_s
    for k in range(1, int(counts.max(initial=1))):
        mk = rank == k
        if not mk.any():
            continue
        u_k = inverse[mk]
        pm = pos_in_me[u_k]
        breg = batch_of[u_k] // nbr
        bloc = batch_of[u_k] % nbr
        exf[pm % 128, breg * g.npair + pair_base[bloc] + (k - 1)] = feats_s[mk]

    # --- per-region r0 in partition-major layout: row = p*(ns_r+1) + s_local
    # (p = row-in-slot, s_local = slot within region, slot ns_r = dump).
    # This makes the V load one contiguous ~(ns_r*C*2)B run per partition.
    m0 = rank == 0
    u0 = inverse[m0]
    for reg, (lo_s, hi_s) in enumerate(g.reg_bounds):
        ns_r = hi_s - lo_s
        r0r = np.zeros(((ns_r + 1) * 128, C), np.float16)
        sel_u = (slot_of_uniq[u0] >= lo_s) & (slot_of_uniq[u0] < hi_s)
        uu = u0[sel_u]
        rows = row_in_slot[uu] * (ns_r + 1) + (slot_of_uniq[uu] - lo_s)
        r0r[rows] = feats_s[m0][sel_u]
        shard[f"r0_{reg}"] = r0r

    # --- fold init = duplicated root rows (zeros for dump lanes).
    fi = np.zeros((128, nbr * g.NREG, C), np.float16)
    for reg in range(g.NREG):
        for bl in range(nbr):
            b = reg * nbr + bl
            fi[:, b, :] = shard[f"r0_{reg}"][exi[:, b]]

    # --- selrow: pair k on partition k%32, chunk k//32; value = root row of
    # the cell within its slot, or 300 (never matches iota 0..127).
    PW = 2 * g.slot_cells
    nchunk = math.ceil(g.npairs / 32)
    selrow = np.full((32, nchunk, PW), 300.0, np.float16)
    kpair = slot_of_uniq // 2
    col = (slot_of_uniq % 2) * g.slot_cells + uniq % g.slot_cells
    selrow[kpair % 32, kpair // 32, col] = row_in_slot
    shard.update({"exi": exi, "exf": exf, "fi": fi, "selrow": selrow})
    return shard


def prep_inputs(g: Geo, features, coordinates, conv_w, gamma, beta):
    feats = np.asarray(features, np.float32).astype(np.float16)
    coords = np.asarray(coordinates)
    b, y, x = coords[:, 0], coords[:, 2], coords[:, 3]
    strip = y // g.ystrip
    wt = np.ascontiguousarray(np.asarray(conv_w, np.float32).T).astype(
        np.float16)                                                 # [C, O]
    gam = np.ascontiguousarray(
        np.asarray(gamma, np.float32).reshape(g.O // 128, 128).T)   # [128, O/128]
    bet = np.ascontiguousarray(
        np.asarray(beta, np.float32).reshape(g.O // 128, 128).T)
    iota = np.arange(128, dtype=np.float32).reshape(128, 1)
    esel = np.zeros((32, 32, 128), np.float16)   # esel[r, i, :] = (r == i)
    esel[np.arange(32), np.arange(32), :] = 1.0
    in_maps = []
    for core in range(g.ncores):
        bb, st = divmod(core, g.NSTRIP)
        m = (b == bb) & (strip == st)
        cell = (y[m] - st * g.ystrip) * g.W + x[m]
        shard = prep_shard(g, feats[m], cell.astype(np.int64))
        shard.update({"wt": wt, "gamma": gam, "beta": bet, "iota": iota,
                      "esel": esel})
        in_maps.append(shard)
    return in_maps


# --------------------------------------------------------------------------
# device program
# --------------------------------------------------------------------------

def build_program(g: Geo) -> bass.Bass:
    C, O = g.C, g.O
    OCH = O // 128
    NS = g.nslot
    SC = g.slot_cells
    PW = 2 * SC
    NPAIR = g.npairs
    NCHUNK = math.ceil(NPAIR / 32)
    lvls = g.lvls
    pair_base = [0]
    for l in lvls[:-1]:
        pair_base.append(pair_base[-1] + l)
    reg_bounds = g.reg_bounds
    NBT = g.NB * g.NREG
    PBN = min(g.PBN, NPAIR)

    nc = bacc.Bacc(num_devices=g.ncores)
    r0_d = [
        nc.declare_dram_parameter(
            f"r0_{r}", [(hi - lo + 1) * 128, C], F16, False)
        for r, (lo, hi) in enumerate(reg_bounds)
    ]
    exi_d = nc.declare_dram_parameter("exi", [128, NBT], I32, False)
    exf_d = nc.declare_dram_parameter("exf", [128, g.npair * g.NREG, C], F16, False)
    fi_d = nc.declare_dram_parameter("fi", [128, NBT, C], F16, False)
    selrow_d = nc.declare_dram_parameter("selrow", [32, NCHUNK, PW], F16, False)
    wt_d = nc.declare_dram_parameter("wt", [C, O], F16, False)
    gam_d = nc.declare_dram_parameter("gamma", [128, OCH], F32, False)
    bet_d = nc.declare_dram_parameter("beta", [128, OCH], F32, False)
    iota_d = nc.declare_dram_parameter("iota", [128, 1], F32, False)
    esel_d = nc.declare_dram_parameter("esel", [32, 32, 128], F16, False)
    out_d = nc.declare_dram_parameter("out", [O, g.cells], F16, True)

    cc_in = nc.dram_tensor("cc_in", [128, 2 * OCH], F32)
    cc_out = nc.dram_tensor("cc_out", [128, 2 * OCH], F32, addr_space="Shared")

    MAX = mybir.AluOpType.max

    with tile.TileContext(nc) as tc:
        with (
            tc.tile_pool(name="vstore", bufs=1) as vstore,
            tc.tile_pool(name="singles", bufs=1) as singles,
            tc.tile_pool(name="fold", bufs=2) as fold,
            tc.tile_pool(name="selp", bufs=3) as selp,
            tc.tile_pool(name="gtp", bufs=2) as gtp,
            tc.tile_pool(name="pbn", bufs=PBN) as pbnp,
            tc.tile_pool(name="osb", bufs=4) as opool,
            tc.tile_pool(name="pstat", bufs=1, space="PSUM") as pstat,
            tc.tile_pool(name="pbrd", bufs=2, space="PSUM") as pbrd,
            tc.tile_pool(name="pgt", bufs=2, space="PSUM") as pgt,
            tc.tile_pool(name="pf", bufs=2, space="PSUM") as pf,
        ):
            # ---- small inputs
            wt16 = singles.tile([C, O], F16)
            nc.sync.dma_start(out=wt16[:], in_=wt_d[:, :])
            wt32 = singles.tile([C, O], F32)
            nc.vector.tensor_copy(out=wt32[:], in_=wt16[:])  # exact fp16->f32
            gam_sb = singles.tile([128, OCH], F32)
            nc.sync.dma_start(out=gam_sb[:], in_=gam_d[:, :])
            bet_sb = singles.tile([128, OCH], F32)
            nc.sync.dma_start(out=bet_sb[:], in_=bet_d[:, :])
            iota_sb = singles.tile([128, 1], F32)
            nc.sync.dma_start(out=iota_sb[:], in_=iota_d[:, :])
            exi_sb = singles.tile([128, NBT], I32)
            nc.scalar.dma_start(out=exi_sb[:], in_=exi_d[:, :])
            exf_sb = singles.tile([128, g.npair * g.NREG, C], F16)
            nc.gpsimd.dma_start(out=exf_sb[:], in_=exf_d[:, :, :])
            fi_sb = singles.tile([128, NBT, C], F16)
            nc.scalar.dma_start(out=fi_sb[:], in_=fi_d[:, :, :])
            selrow_sb = singles.tile([32, NCHUNK, PW], F16)
            nc.sync.dma_start(out=selrow_sb[:], in_=selrow_d[:, :, :])
            # esel[r, i, m] = (r == i): lhsT that broadcasts rhs row i.
            esel = singles.tile([32, 32, 128], F16)
            nc.sync.dma_start(out=esel[:], in_=esel_d[:, :, :])
            ones_f32 = singles.tile([128, 1], F32)
            nc.vector.memset(ones_f32[:], 1.0)
            eps_t = singles.tile([128, 1], F32)
            nc.vector.memset(eps_t[:], float(g.EPS))

            # ---- per-region: fold collisions into r0 in place, then load V.
            v_r = []
            for reg, (lo_s, hi_s) in enumerate(reg_bounds):
                ns_r = hi_s - lo_s
                if ns_r <= 0:
                    v_r.append(None)
                    continue
                for bl in range(g.NB):
                    b = reg * g.NB + bl
                    ft = fold.tile([128, C], F16, tag="fold")
                    nc.vector.tensor_tensor(
                        out=ft[:], in0=fi_sb[:, b, :],
                        in1=exf_sb[:, reg * g.npair + pair_base[bl], :],
                        op=MAX,
                    )
                    for l in range(1, lvls[bl]):
                        nc.vector.tensor_tensor(
                            out=ft[:], in0=ft[:],
                            in1=exf_sb[:, reg * g.npair + pair_base[bl] + l, :],
                            op=MAX,
                        )
                    nc.gpsimd.indirect_dma_start(
                        out=r0_d[reg][:, :],
                        out_offset=bass.IndirectOffsetOnAxis(
                            ap=exi_sb[:, b : b + 1], axis=0
                        ),
                        in_=ft[:], in_offset=None,
                    )
                vt = vstore.tile([128, ns_r, C + 1], F16, tag=f"v{reg}")
                nc.vector.memset(vt[:, :, C : C + 1], 1.0)
                c3 = r0_d[reg].ap().rearrange("(p s) c -> p s c", s=ns_r + 1)
                ld_eng = [nc.sync, nc.scalar, nc.sync, nc.scalar][reg % 4]
                ld_eng.dma_start(out=vt[:, :, :C], in_=c3[:, :ns_r, :])
                v_r.append(vt)

            # ---- sigma = sum_s V_s^T [V_s | 1]  -> [Sigma | sv]
            sig_ps = pstat.tile([128, C + 1], F32, space="PSUM", tag="sig")
            si = 0
            for reg, (lo_s, hi_s) in enumerate(reg_bounds):
                vt = v_r[reg]
                for sl in range(hi_s - lo_s):
                    nc.tensor.matmul(
                        out=sig_ps[:],
                        lhsT=vt[:, sl, :C],
                        rhs=vt[:, sl, :],
                        start=(si == 0), stop=(si == NS - 1),
                    )
                    si += 1
            sig_sb = singles.tile([128, C + 1], F32)
            nc.vector.tensor_copy(out=sig_sb[:], in_=sig_ps[:])

            # ---- local projected moments: q_o = w_o^T Sigma w_o, m_o = w_o.sv
            proj = pstat.tile([128, O + 2 * OCH], F32, space="PSUM", tag="proj")
            nc.tensor.matmul(
                out=proj[:, :O], lhsT=sig_sb[:, :C], rhs=wt32[:],
                start=True, stop=True,
            )
            u_sb = singles.tile([128, O], F32)
            nc.vector.tensor_tensor(
                out=u_sb[:], in0=proj[:, :O], in1=wt32[:],
                op=mybir.AluOpType.mult,
            )
            for ch in range(OCH):
                nc.tensor.matmul(
                    out=proj[:, O + ch : O + ch + 1],
                    lhsT=u_sb[:, ch * 128 : (ch + 1) * 128],
                    rhs=ones_f32[:], start=True, stop=True,
                )
                nc.tensor.matmul(
                    out=proj[:, O + OCH + ch : O + OCH + ch + 1],
                    lhsT=wt32[:, ch * 128 : (ch + 1) * 128],
                    rhs=sig_sb[:, C : C + 1], start=True, stop=True,
                )
            red_sb = singles.tile([128, 2 * OCH], F32)
            nc.vector.tensor_copy(out=red_sb[:], in_=proj[:, O : O + 2 * OCH])
            nc.sync.dma_start(out=cc_in[:, :], in_=red_sb[:])
            nc.gpsimd.collective_compute(
                "AllReduce",
                mybir.AluOpType.add,
                replica_groups=[list(range(g.ncores))],
                ins=[cc_in.ap().opt()],
                outs=[cc_out.ap().opt()],
            )

            # ---- phase C producers for the first PBN pairs (independent of
            # the collective; emitted before the BN math so the PE/DVE/GPSIMD
            # queues keep flowing while the AllReduce is in flight).
            slot_loc = []            # slot -> (region vt, local index)
            for reg, (lo_s, hi_s) in enumerate(reg_bounds):
                for sl in range(hi_s - lo_s):
                    slot_loc.append((v_r[reg], sl))

            def emit_pair_producers(k):
                base = k * PW
                w = min(PW, g.cells - base)
                s0 = 2 * k
                brd_ps = pbrd.tile([128, PW], F32, space="PSUM", tag="brd")
                nc.tensor.matmul(
                    out=brd_ps[:],
                    lhsT=esel[:, k % 32, :],
                    rhs=selrow_sb[:, k // 32, :],
                    start=True, stop=True,
                )
                sel16 = selp.tile([128, PW], F16, tag="sel")
                nc.vector.tensor_scalar(
                    sel16[:], brd_ps[:], iota_sb[:, 0:1], None,
                    mybir.AluOpType.is_equal,
                )
                gt_ps = pgt.tile([128, PW], F32, space="PSUM", tag="gt")
                w0 = min(SC, w)
                vt0, sl0 = slot_loc[s0]
                nc.tensor.matmul(
                    out=gt_ps[:, :w0],
                    lhsT=vt0[:, sl0, :C],
                    rhs=sel16[:, :w0],
                    start=True, stop=True,
                )
                if w > SC:
                    vt1, sl1 = slot_loc[s0 + 1]
                    nc.tensor.matmul(
                        out=gt_ps[:, SC:w],
                        lhsT=vt1[:, sl1, :C],
                        rhs=sel16[:, SC:w],
                        start=True, stop=True,
                    )
                gt16 = gtp.tile([128, PW], F16, tag="gt16")
                nc.vector.tensor_copy(out=gt16[:, :w], in_=gt_ps[:, :w])
                fps = []
                for ch in range(OCH):
                    fp_ps = pf.tile([128, PW], F32, space="PSUM", tag="fp")
                    nc.tensor.matmul(
                        out=fp_ps[:, :w],
                        lhsT=wt16[:, ch * 128 : (ch + 1) * 128],
                        rhs=gt16[:, :w],
                        start=True, stop=True,
                    )
                    fps.append(fp_ps)
                return w, base, fps

            def emit_pair_act(k, w, base, srcs, a_t, b_t, use_gpsimd=False):
                for ch in range(OCH):
                    ot = opool.tile([128, PW], F16, tag=f"ot{ch}")
                    if use_gpsimd:
                        # gpsimd cannot touch PSUM but the buffered pairs read
                        # SBUF; 2-op BN+ReLU keeps the ACT engine free.
                        nc.gpsimd.tensor_scalar(
                            ot[:, :w], srcs[ch],
                            a_t[:, ch : ch + 1], b_t[:, ch : ch + 1],
                            mybir.AluOpType.mult, mybir.AluOpType.add,
                        )
                        nc.gpsimd.tensor_scalar(
                            ot[:, :w], ot[:, :w], 0.0, None,
                            mybir.AluOpType.max,
                        )
                    else:
                        nc.scalar.activation(
                            out=ot[:, :w], in_=srcs[ch],
                            func=mybir.ActivationFunctionType.Relu,
                            scale=a_t[:, ch : ch + 1],
                            bias=b_t[:, ch : ch + 1],
                        )
                    eng = nc.sync if k % 2 == 0 else nc.scalar
                    eng.dma_start(
                        out=out_d[ch * 128 : (ch + 1) * 128, base : base + w],
                        in_=ot[:, :w],
                    )

            deferred = []
            for k in range(PBN):
                w, base, fps = emit_pair_producers(k)
                pb = pbnp.tile([128, 2 * PW], F16, tag="pbn")
                for ch in range(OCH):
                    nc.vector.tensor_copy(
                        out=pb[:, ch * PW : ch * PW + w], in_=fps[ch][:, :w]
                    )
                deferred.append((k, w, base, pb))

            # ---- BN constants (waits on the collective)
            mom_raw = singles.tile([128, 2 * OCH], F32)
            nc.sync.dma_start(out=mom_raw[:], in_=cc_out[:, :])
            mom = singles.tile([128, 2 * OCH], F32)      # [q/N | mean]
            nc.scalar.mul(out=mom[:], in_=mom_raw[:], mul=1.0 / float(g.ncell_total))
            var_t = singles.tile([128, OCH], F32)
            nc.vector.tensor_tensor(
                out=var_t[:], in0=mom[:, OCH:], in1=mom[:, OCH:],
                op=mybir.AluOpType.mult,
            )
            nc.vector.tensor_tensor(
                out=var_t[:], in0=mom[:, :OCH], in1=var_t[:],
                op=mybir.AluOpType.subtract,
            )
            rstd = singles.tile([128, OCH], F32)
            nc.scalar.activation(
                out=rstd[:], in_=var_t[:],
                func=mybir.ActivationFunctionType.Sqrt, bias=eps_t[:],
            )
            nc.vector.reciprocal(out=rstd[:], in_=rstd[:])
            a_t = singles.tile([128, OCH], F32)
            nc.vector.tensor_tensor(
                out=a_t[:], in0=gam_sb[:], in1=rstd[:], op=mybir.AluOpType.mult
            )
            b_t = singles.tile([128, OCH], F32)
            nc.vector.tensor_tensor(
                out=b_t[:], in0=mom[:, OCH:], in1=a_t[:], op=mybir.AluOpType.mult
            )
            nc.vector.tensor_tensor(
                out=b_t[:], in0=bet_sb[:], in1=b_t[:], op=mybir.AluOpType.subtract
            )

            # ---- deferred ACT + store for the buffered pairs (alternating
            # ACT engine / gpsimd so neither becomes the serial tail)
            for k, w, base, pb in deferred:
                srcs = [pb[:, ch * PW : ch * PW + w] for ch in range(OCH)]
                emit_pair_act(k, w, base, srcs, a_t, b_t, use_gpsimd=(k % 2 == 1))

            # ---- remaining pairs stream straight from PSUM
            for k in range(PBN, NPAIR):
                w, base, fps = emit_pair_producers(k)
                srcs = [fps[ch][:, :w] for ch in range(OCH)]
                emit_pair_act(k, w, base, srcs, a_t, b_t)
    return nc


_PROGRAM_CACHE: dict = {}


def get_program(g: Geo) -> bass.Bass:
    if g not in _PROGRAM_CACHE:
        nc = build_program(g)
        nc.finalize()
        _PROGRAM_CACHE[g] = nc
    return _PROGRAM_CACHE[g]


def assemble_output(g: Geo, per_core: list) -> np.ndarray:
    out = np.empty((g.B, g.O, g.H, g.W), np.float32)
    for core in range(g.ncores):
        bb, st = divmod(core, g.NSTRIP)
        out[bb, :, st * g.ystrip : (st + 1) * g.ystrip, :] = (
            np.asarray(per_core[core], np.float32).reshape(g.O, g.ystrip, g.W)
        )
    return out


def kernel(features, coordinates, conv_w, gamma, beta):
    g = GEO
    in_maps = prep_inputs(g, features, coordinates, conv_w, gamma, beta)
    nc = get_program(g)
    res = run_bass_kernel_spmd(nc, in_maps, core_ids=list(range(g.ncores)))
    return assemble_output(g, [r["out"] for r in res.results])


# revision 20
# speedup vs baseline: 1.8886x; 1.8886x over previous
"""BEV feature extractor (scatter-max -> 1x1 conv -> BN(train) -> ReLU) on 8 TRN2 cores.

Sharding: data-parallel over (batch, y-strip) -> 8 shards, BN stats all-reduced.

v1 design (fp16 data paths, ~3x less HBM traffic + 4x PE rate vs the f32 baseline):

  1. Host packs each shard: occupied cells of SLOT_BLKS consecutive 128-cell
     blocks form one 128-row *slot*; root (first) points go into the per-region
     r0 tensors (fp16). Colliding extra points go into fold batches of 128
     (exf), with the root rows duplicated alongside (fi) so no device gather is
     needed. A per-batch level schedule bounds collision depth.
  2. Device folds: f = max(fi, exf levels) on DVE, then indirect-scatters f
     back into r0 *in place* (region-split r0 keeps the 4 fold chains
     independent). V tiles [128, slots, C+1] (fp16, fused ones column) load
     straight from the folded r0 -- no DRAM->DRAM comb copy.
  3. PE accumulates sig = sum_s V_s^T [V_s | 1] (fp16 in, f32 PSUM), projects
     the per-core moments q_o = w_o^T Sigma w_o, m_o = w_o . sv locally, and a
     tiny [128, 2*OCH] AllReduce(+) produces global BN stats:
     mean = m/N, var = q/N - mean^2, a = gamma/sqrt(var+eps), b = beta-mean*a.
  4. Phase C per slot-pair: the 0/1 selection matrix is rebuilt on device from
     a small row-index tensor (selrow) via a K=32 broadcast matmul + DVE
     is_equal (kills the 20MB/core sel load of the baseline); GT = V_s^T @ Sel
     densifies+transposes; conv = W^T_chunk @ GT; ACT applies relu(x*a+b) and
     the result streams out as fp16 (halves the output write).
     The first PBN pairs buffer their conv output in SBUF (fp16) so PE/DVE run
     through the AllReduce window; their ACT+store is emitted after the BN
     constants so only the scalar engine waits on the collective.
"""

import math
from dataclasses import dataclass

import numpy as np

import concourse.bass as bass
import concourse.tile as tile
from concourse import bacc, mybir
from concourse.bass_utils import run_bass_kernel_spmd

F32 = mybir.dt.float32
F16 = mybir.dt.float16
I32 = mybir.dt.int32


@dataclass(frozen=True)
class Geo:
    B: int = 2
    H: int = 400
    W: int = 400
    C: int = 128            # input channels (= partition count)
    O: int = 256            # output channels (multiple of 128)
    NSTRIP: int = 4         # y-strips per batch; B*NSTRIP = 8 cores
    SLOT_BLKS: int = 2      # 128-cell blocks packed per 128-row slot
    NB: int = 6             # fold batches per region (128 roots each)
    NREG: int = 4           # slot regions (independent r0 tensors)
    LVLS: tuple = (5, 2)    # per-batch fold depth; batches beyond get depth 1
    PBN: int = 56           # pairs whose conv output is SBUF-buffered pre-BN
    EPS: float = 1e-5

    @property
    def ystrip(self):
        return self.H // self.NSTRIP

    @property
    def cells(self):
        return self.ystrip * self.W

    @property
    def ncores(self):
        return self.B * self.NSTRIP

    @property
    def slot_cells(self):
        return 128 * self.SLOT_BLKS

    @property
    def nslot(self):
        return math.ceil(self.cells / self.slot_cells)

    @property
    def npairs(self):
        return math.ceil(self.nslot / 2)

    @property
    def lvls(self):
        return tuple(self.LVLS) + (1,) * (self.NB - len(self.LVLS))

    @property
    def npair(self):                 # (batch, level) pairs
        return sum(self.lvls)

    @property
    def ncell_total(self):
        return self.B * self.H * self.W

    @property
    def rs(self):                    # slots per region
        return math.ceil(self.nslot / self.NREG)

    @property
    def reg_bounds(self):
        out = []
        for reg in range(self.NREG):
            lo = min(reg * self.rs, self.nslot)
            hi = self.nslot if reg == self.NREG - 1 else min(
                (reg + 1) * self.rs, self.nslot)
            out.append((lo, hi))
        return out


GEO = Geo()


# --------------------------------------------------------------------------
# host-side shard prep
# --------------------------------------------------------------------------

def prep_shard(g: Geo, feats: np.ndarray, cell: np.ndarray) -> dict:
    """feats [n, C] fp16, cell [n] int in [0, g.cells)."""
    C = g.C
    order = np.argsort(cell, kind="stable")
    cell_s = cell[order]
    feats_s = feats[order]
    uniq, seg_start, inverse, counts = np.unique(
        cell_s, return_index=True, return_inverse=True, return_counts=True
    )
    rank = np.arange(len(cell_s)) - seg_start[inverse]

    # --- slot packing: cell j -> slot j // slot_cells; occupied cells of a
    # slot occupy consecutive rows (cell order) within the slot's 128 rows.
    slot_of_uniq = uniq // g.slot_cells
    occ_per_slot = np.zeros(g.nslot, np.int64)
    np.add.at(occ_per_slot, slot_of_uniq, 1)
    assert occ_per_slot.max(initial=0) <= 128, (
        f"slot overflow: {occ_per_slot.max()}"
    )
    first_of_slot = np.zeros(g.nslot, np.int64)
    first_of_slot[1:] = np.cumsum(occ_per_slot)[:-1]
    row_in_slot = np.arange(len(uniq)) - first_of_slot[slot_of_uniq]
    rowid = slot_of_uniq * 128 + row_in_slot          # global packed row

    lvls = g.lvls
    nbr = len(lvls)
    pair_base = np.cumsum((0,) + lvls[:-1])
    exi = np.zeros((128, nbr * g.NREG), np.int32)
    exf = np.zeros((128, g.npair * g.NREG, C), np.float16)
    pos_in_me = np.zeros(len(uniq), np.int64)
    batch_of = np.zeros(len(uniq), np.int64)
    reg_lo = np.zeros(len(uniq), np.int64)
    shard = {}
    for reg, (lo_s, hi_s) in enumerate(g.reg_bounds):
        ns_r = hi_s - lo_s
        exi[:, reg * nbr : (reg + 1) * nbr] = (
            np.arange(128)[:, None] * (ns_r + 1) + ns_r   # per-lane dump slot
        )
        inreg = (counts > 1) & (slot_of_uniq >= lo_s) & (slot_of_uniq < hi_s)
        ord_me = np.argsort(-counts[inreg], kind="stable")
        me_uniq = np.flatnonzero(inreg)[ord_me]
        nme = len(me_uniq)
        assert nme <= 128 * nbr, f"region fold capacity exceeded: {nme}"
        bi = np.arange(nme) // 128
        pi = np.arange(nme) % 128
        assert (counts[me_uniq] - 1 <= np.asarray(lvls)[bi]).all(), (
            "collision depth exceeds fold schedule"
        )
        exi[pi, reg * nbr + bi] = (
            row_in_slot[me_uniq] * (ns_r + 1) + (slot_of_uniq[me_uniq] - lo_s)
        ).astype(np.int32)
        pos_in_me[me_uniq] = np.arange(nme)
        batch_of[me_uniq] = reg * nbr + bi
        reg_lo[me_uniq] = lo_s
    for k in range(1, int(counts.max(initial=1))):
        mk = rank == k
        if not mk.any():
            continue
        u_k = inverse[mk]
        pm = pos_in_me[u_k]
        breg = batch_of[u_k] // nbr
        bloc = batch_of[u_k] % nbr
        exf[pm % 128, breg * g.npair + pair_base[bloc] + (k - 1)] = feats_s[mk]

    # --- per-region r0 in partition-major layout: row = p*(ns_r+1) + s_local
    # (p = row-in-slot, s_local = slot within region, slot ns_r = dump).
    # This makes the V load one contiguous ~(ns_r*C*2)B run per partition.
    m0 = rank == 0
    u0 = inverse[m0]
    for reg, (lo_s, hi_s) in enumerate(g.reg_bounds):
        ns_r = hi_s - lo_s
        r0r = np.zeros(((ns_r + 1) * 128, C), np.float16)
        sel_u = (slot_of_uniq[u0] >= lo_s) & (slot_of_uniq[u0] < hi_s)
        uu = u0[sel_u]
        rows = row_in_slot[uu] * (ns_r + 1) + (slot_of_uniq[uu] - lo_s)
        r0r[rows] = feats_s[m0][sel_u]
        shard[f"r0_{reg}"] = r0r

    # --- fold init = duplicated root rows (zeros for dump lanes).
    fi = np.zeros((128, nbr * g.NREG, C), np.float16)
    for reg in range(g.NREG):
        for bl in range(nbr):
            b = reg * nbr + bl
            fi[:, b, :] = shard[f"r0_{reg}"][exi[:, b]]

    # --- selrow: pair k on partition k%32, chunk k//32; value = root row of
    # the cell within its slot, or 300 (never matches iota 0..127).
    PW = 2 * g.slot_cells
    nchunk = math.ceil(g.npairs / 32)
    selrow = np.full((32, nchunk, PW), 300.0, np.float16)
    kpair = slot_of_uniq // 2
    col = (slot_of_uniq % 2) * g.slot_cells + uniq % g.slot_cells
    selrow[kpair % 32, kpair // 32, col] = row_in_slot
    shard.update({"exi": exi, "exf": exf, "fi": fi, "selrow": selrow})
    return shard


def prep_inputs(g: Geo, features, coordinates, conv_w, gamma, beta):
    feats = np.asarray(features, np.float32).astype(np.float16)
    coords = np.asarray(coordinates)
    b, y, x = coords[:, 0], coords[:, 2], coords[:, 3]
    strip = y // g.ystrip
    wt = np.ascontiguousarray(np.asarray(conv_w, np.float32).T).astype(
        np.float16)                                                 # [C, O]
    gam = np.ascontiguousarray(
        np.asarray(gamma, np.float32).reshape(g.O // 128, 128).T)   # [128, O/128]
    bet = np.ascontiguousarray(
        np.asarray(beta, np.float32).reshape(g.O // 128, 128).T)
    iota = np.arange(128, dtype=np.float32).reshape(128, 1)
    esel = np.zeros((32, 32, 128), np.float16)   # esel[r, i, :] = (r == i)
    esel[np.arange(32), np.arange(32), :] = 1.0
    in_maps = []
    for core in range(g.ncores):
        bb, st = divmod(core, g.NSTRIP)
        m = (b == bb) & (strip == st)
        cell = (y[m] - st * g.ystrip) * g.W + x[m]
        shard = prep_shard(g, feats[m], cell.astype(np.int64))
        shard.update({"wt": wt, "gamma": gam, "beta": bet, "iota": iota,
                      "esel": esel})
        in_maps.append(shard)
    return in_maps


# --------------------------------------------------------------------------
# device program
# --------------------------------------------------------------------------

def build_program(g: Geo) -> bass.Bass:
    C, O = g.C, g.O
    OCH = O // 128
    NS = g.nslot
    SC = g.slot_cells
    PW = 2 * SC
    NPAIR = g.npairs
    NCHUNK = math.ceil(NPAIR / 32)
    lvls = g.lvls
    pair_base = [0]
    for l in lvls[:-1]:
        pair_base.append(pair_base[-1] + l)
    reg_bounds = g.reg_bounds
    NBT = g.NB * g.NREG
    PBN = min(g.PBN, NPAIR)

    nc = bacc.Bacc(num_devices=g.ncores)
    r0_d = [
        nc.declare_dram_parameter(
            f"r0_{r}", [(hi - lo + 1) * 128, C], F16, False)
        for r, (lo, hi) in enumerate(reg_bounds)
    ]
    exi_d = nc.declare_dram_parameter("exi", [128, NBT], I32, False)
    exf_d = nc.declare_dram_parameter("exf", [128, g.npair * g.NREG, C], F16, False)
    fi_d = nc.declare_dram_parameter("fi", [128, NBT, C], F16, False)
    selrow_d = nc.declare_dram_parameter("selrow", [32, NCHUNK, PW], F16, False)
    wt_d = nc.declare_dram_parameter("wt", [C, O], F16, False)
    gam_d = nc.declare_dram_parameter("gamma", [128, OCH], F32, False)
    bet_d = nc.declare_dram_parameter("beta", [128, OCH], F32, False)
    iota_d = nc.declare_dram_parameter("iota", [128, 1], F32, False)
    esel_d = nc.declare_dram_parameter("esel", [32, 32, 128], F16, False)
    out_d = nc.declare_dram_parameter("out", [O, g.cells], F16, True)

    cc_in = nc.dram_tensor("cc_in", [128, 2 * OCH], F32)
    cc_out = nc.dram_tensor("cc_out", [128, 2 * OCH], F32, addr_space="Shared")

    MAX = mybir.AluOpType.max

    with tile.TileContext(nc) as tc:
        with (
            tc.tile_pool(name="vstore", bufs=1) as vstore,
            tc.tile_pool(name="singles", bufs=1) as singles,
            tc.tile_pool(name="fold", bufs=2) as fold,
            tc.tile_pool(name="selp", bufs=3) as selp,
            tc.tile_pool(name="gtp", bufs=2) as gtp,
            tc.tile_pool(name="pbn", bufs=PBN) as pbnp,
            tc.tile_pool(name="osb", bufs=4) as opool,
            tc.tile_pool(name="pstat", bufs=1, space="PSUM") as pstat,
            tc.tile_pool(name="pbrd", bufs=2, space="PSUM") as pbrd,
            tc.tile_pool(name="pgt", bufs=2, space="PSUM") as pgt,
            tc.tile_pool(name="pf", bufs=2, space="PSUM") as pf,
        ):
            # ---- small inputs
            wt16 = singles.tile([C, O], F16)
            nc.sync.dma_start(out=wt16[:], in_=wt_d[:, :])
            wt32 = singles.tile([C, O], F32)
            nc.vector.tensor_copy(out=wt32[:], in_=wt16[:])  # exact fp16->f32
            gam_sb = singles.tile([128, OCH], F32)
            nc.sync.dma_start(out=gam_sb[:], in_=gam_d[:, :])
            bet_sb = singles.tile([128, OCH], F32)
            nc.sync.dma_start(out=bet_sb[:], in_=bet_d[:, :])
            iota_sb = singles.tile([128, 1], F32)
            nc.sync.dma_start(out=iota_sb[:], in_=iota_d[:, :])
            exi_sb = singles.tile([128, NBT], I32)
            nc.scalar.dma_start(out=exi_sb[:], in_=exi_d[:, :])
            exf_sb = singles.tile([128, g.npair * g.NREG, C], F16)
            nc.gpsimd.dma_start(out=exf_sb[:], in_=exf_d[:, :, :])
            fi_sb = singles.tile([128, NBT, C], F16)
            nc.scalar.dma_start(out=fi_sb[:], in_=fi_d[:, :, :])
            selrow_sb = singles.tile([32, NCHUNK, PW], F16)
            nc.sync.dma_start(out=selrow_sb[:], in_=selrow_d[:, :, :])
            # esel[r, i, m] = (r == i): lhsT that broadcasts rhs row i.
            esel = singles.tile([32, 32, 128], F16)
            nc.sync.dma_start(out=esel[:], in_=esel_d[:, :, :])
            ones_f32 = singles.tile([128, 1], F32)
            nc.vector.memset(ones_f32[:], 1.0)
            eps_t = singles.tile([128, 1], F32)
            nc.vector.memset(eps_t[:], float(g.EPS))

            # ---- per-region: fold collisions into r0 in place, then load V.
            v_r = []
            for reg, (lo_s, hi_s) in enumerate(reg_bounds):
                ns_r = hi_s - lo_s
                if ns_r <= 0:
                    v_r.append(None)
                    continue
                for bl in range(g.NB):
                    b = reg * g.NB + bl
                    ft = fold.tile([128, C], F16, tag="fold")
                    nc.vector.tensor_tensor(
                        out=ft[:], in0=fi_sb[:, b, :],
                        in1=exf_sb[:, reg * g.npair + pair_base[bl], :],
                        op=MAX,
                    )
                    for l in range(1, lvls[bl]):
                        nc.vector.tensor_tensor(
                            out=ft[:], in0=ft[:],
                            in1=exf_sb[:, reg * g.npair + pair_base[bl] + l, :],
                            op=MAX,
                        )
                    nc.gpsimd.indirect_dma_start(
                        out=r0_d[reg][:, :],
                        out_offset=bass.IndirectOffsetOnAxis(
                            ap=exi_sb[:, b : b + 1], axis=0
                        ),
                        in_=ft[:], in_offset=None,
                    )
                vt = vstore.tile([128, ns_r, C + 1], F16, tag=f"v{reg}")
                nc.vector.memset(vt[:, :, C : C + 1], 1.0)
                c3 = r0_d[reg].ap().rearrange("(p s) c -> p s c", s=ns_r + 1)
                ld_eng = [nc.sync, nc.scalar, nc.sync, nc.scalar][reg % 4]
                ld_eng.dma_start(out=vt[:, :, :C], in_=c3[:, :ns_r, :])
                v_r.append(vt)

            # ---- sigma = sum_s V_s^T [V_s | 1]  -> [Sigma | sv]
            sig_ps = pstat.tile([128, C + 1], F32, space="PSUM", tag="sig")
            si = 0
            for reg, (lo_s, hi_s) in enumerate(reg_bounds):
                vt = v_r[reg]
                for sl in range(hi_s - lo_s):
                    nc.tensor.matmul(
                        out=sig_ps[:],
                        lhsT=vt[:, sl, :C],
                        rhs=vt[:, sl, :],
                        start=(si == 0), stop=(si == NS - 1),
                    )
                    si += 1
            sig_sb = singles.tile([128, C + 1], F32)
            nc.vector.tensor_copy(out=sig_sb[:], in_=sig_ps[:])

            # ---- local projected moments: q_o = w_o^T Sigma w_o, m_o = w_o.sv
            proj = pstat.tile([128, O + 2 * OCH], F32, space="PSUM", tag="proj")
            nc.tensor.matmul(
                out=proj[:, :O], lhsT=sig_sb[:, :C], rhs=wt32[:],
                start=True, stop=True,
            )
            u_sb = singles.tile([128, O], F32)
            nc.vector.tensor_tensor(
                out=u_sb[:], in0=proj[:, :O], in1=wt32[:],
                op=mybir.AluOpType.mult,
            )
            for ch in range(OCH):
                nc.tensor.matmul(
                    out=proj[:, O + ch : O + ch + 1],
                    lhsT=u_sb[:, ch * 128 : (ch + 1) * 128],
                    rhs=ones_f32[:], start=True, stop=True,
                )
                nc.tensor.matmul(
                    out=proj[:, O + OCH + ch : O + OCH + ch + 1],
                    lhsT=wt32[:, ch * 128 : (ch + 1) * 128],
                    rhs=sig_sb[:, C : C + 1], start=True, stop=True,
                )
            red_sb = singles.tile([128, 2 * OCH], F32)
            nc.vector.tensor_copy(out=red_sb[:], in_=proj[:, O : O + 2 * OCH])
            nc.sync.dma_start(out=cc_in[:, :], in_=red_sb[:])
            nc.gpsimd.collective_compute(
                "AllReduce",
                mybir.AluOpType.add,
                replica_groups=[list(range(g.ncores))],
                ins=[cc_in.ap().opt()],
                outs=[cc_out.ap().opt()],
            )

            # ---- phase C producers for the first PBN pairs (independent of
            # the collective; emitted before the BN math so the PE/DVE/GPSIMD
            # queues keep flowing while the AllReduce is in flight).
            slot_loc = []            # slot -> (region vt, local index)
            for reg, (lo_s, hi_s) in enumerate(reg_bounds):
                for sl in range(hi_s - lo_s):
                    slot_loc.append((v_r[reg], sl))

            def emit_pair_producers(k):
                base = k * PW
                w = min(PW, g.cells - base)
                s0 = 2 * k
                brd_ps = pbrd.tile([128, PW], F32, space="PSUM", tag="brd")
                nc.tensor.matmul(
                    out=brd_ps[:],
                    lhsT=esel[:, k % 32, :],
                    rhs=selrow_sb[:, k // 32, :],
                    start=True, stop=True,
                )
                sel16 = selp.tile([128, PW], F16, tag="sel")
                nc.vector.tensor_scalar(
                    sel16[:], brd_ps[:], iota_sb[:, 0:1], None,
                    mybir.AluOpType.is_equal,
                )
                gt_ps = pgt.tile([128, PW], F32, space="PSUM", tag="gt")
                w0 = min(SC, w)
                vt0, sl0 = slot_loc[s0]
                nc.tensor.matmul(
                    out=gt_ps[:, :w0],
                    lhsT=vt0[:, sl0, :C],
                    rhs=sel16[:, :w0],
                    start=True, stop=True,
                )
                if w > SC:
                    vt1, sl1 = slot_loc[s0 + 1]
                    nc.tensor.matmul(
                        out=gt_ps[:, SC:w],
                        lhsT=vt1[:, sl1, :C],
                        rhs=sel16[:, SC:w],
                        start=True, stop=True,
                    )
                gt16 = gtp.tile([128, PW], F16, tag="gt16")
                nc.vector.tensor_copy(out=gt16[:, :w], in_=gt_ps[:, :w])
                fps = []
                for ch in range(OCH):
                    fp_ps = pf.tile([128, PW], F32, space="PSUM", tag="fp")
                    nc.tensor.matmul(
                        out=fp_ps[:, :w],
                        lhsT=wt16[:, ch * 128 : (ch + 1) * 128],
                        rhs=gt16[:, :w],
                        start=True, stop=True,
                    )
                    fps.append(fp_ps)
                return w, base, fps

            def emit_pair_act(k, w, base, srcs, a_t, b_t, use_gpsimd=False):
                for ch in range(OCH):
                    ot = opool.tile([128, PW], F16, tag=f"ot{ch}")
                    if use_gpsimd:
                        # gpsimd cannot touch PSUM but the buffered pairs read
                        # SBUF; 2-op BN+ReLU keeps the ACT engine free.
                        nc.gpsimd.tensor_scalar(
                            ot[:, :w], srcs[ch],
                            a_t[:, ch : ch + 1], b_t[:, ch : ch + 1],
                            mybir.AluOpType.mult, mybir.AluOpType.add,
                        )
                        nc.gpsimd.tensor_scalar(
                            ot[:, :w], ot[:, :w], 0.0, None,
                            mybir.AluOpType.max,
                        )
                    else:
                        nc.scalar.activation(
                            out=ot[:, :w], in_=srcs[ch],
                            func=mybir.ActivationFunctionType.Relu,
                            scale=a_t[:, ch : ch + 1],
                            bias=b_t[:, ch : ch + 1],
                        )
                    eng = nc.sync if k % 2 == 0 else nc.scalar
                    eng.dma_start(
                        out=out_d[ch * 128 : (ch + 1) * 128, base : base + w],
                        in_=ot[:, :w],
                    )

            deferred = []
            for k in range(PBN):
                w, base, fps = emit_pair_producers(k)
                pb = pbnp.tile([128, 2 * PW], F16, tag="pbn")
                for ch in range(OCH):
                    nc.vector.tensor_copy(
                        out=pb[:, ch * PW : ch * PW + w], in_=fps[ch][:, :w]
                    )
                deferred.append((k, w, base, pb))

            # ---- BN constants (waits on the collective)
            mom_raw = singles.tile([128, 2 * OCH], F32)
            nc.sync.dma_start(out=mom_raw[:], in_=cc_out[:, :])
            mom = singles.tile([128, 2 * OCH], F32)      # [q/N | mean]
            nc.scalar.mul(out=mom[:], in_=mom_raw[:], mul=1.0 / float(g.ncell_total))
            var_t = singles.tile([128, OCH], F32)
            nc.vector.tensor_tensor(
                out=var_t[:], in0=mom[:, OCH:], in1=mom[:, OCH:],
                op=mybir.AluOpType.mult,
            )
            nc.vector.tensor_tensor(
                out=var_t[:], in0=mom[:, :OCH], in1=var_t[:],
                op=mybir.AluOpType.subtract,
            )
            rstd = singles.tile([128, OCH], F32)
            nc.scalar.activation(
                out=rstd[:], in_=var_t[:],
                func=mybir.ActivationFunctionType.Sqrt, bias=eps_t[:],
            )
            nc.vector.reciprocal(out=rstd[:], in_=rstd[:])
            a_t = singles.tile([128, OCH], F32)
            nc.vector.tensor_tensor(
                out=a_t[:], in0=gam_sb[:], in1=rstd[:], op=mybir.AluOpType.mult
            )
            b_t = singles.tile([128, OCH], F32)
            nc.vector.tensor_tensor(
                out=b_t[:], in0=mom[:, OCH:], in1=a_t[:], op=mybir.AluOpType.mult
            )
            nc.vector.tensor_tensor(
                out=b_t[:], in0=bet_sb[:], in1=b_t[:], op=mybir.AluOpType.subtract
            )

            # ---- deferred ACT + store for the buffered pairs
            for k, w, base, pb in deferred:
                srcs = [pb[:, ch * PW : ch * PW + w] for ch in range(OCH)]
                emit_pair_act(k, w, base, srcs, a_t, b_t)

            # ---- remaining pairs stream straight from PSUM
            for k in range(PBN, NPAIR):
                w, base, fps = emit_pair_producers(k)
                srcs = [fps[ch][:, :w] for ch in range(OCH)]
                emit_pair_act(k, w, base, srcs, a_t, b_t)
    return nc


_PROGRAM_CACHE: dict = {}


def get_program(g: Geo) -> bass.Bass:
    if g not in _PROGRAM_CACHE:
        nc = build_program(g)
        nc.finalize()
        _PROGRAM_CACHE[g] = nc
    return _PROGRAM_CACHE[g]


def assemble_output(g: Geo, per_core: list) -> np.ndarray:
    out = np.empty((g.B, g.O, g.H, g.W), np.float32)
    for core in range(g.ncores):
        bb, st = divmod(core, g.NSTRIP)
        out[bb, :, st * g.ystrip : (st + 1) * g.ystrip, :] = (
            np.asarray(per_core[core], np.float32).reshape(g.O, g.ystrip, g.W)
        )
    return out


def kernel(features, coordinates, conv_w, gamma, beta):
    g = GEO
    in_maps = prep_inputs(g, features, coordinates, conv_w, gamma, beta)
    nc = get_program(g)
    res = run_bass_kernel_spmd(nc, in_maps, core_ids=list(range(g.ncores)))
    return assemble_output(g, [r["out"] for r in res.results])


# revision 29
# speedup vs baseline: 2.1103x; 1.1174x over previous
"""BEV feature extractor (scatter-max -> 1x1 conv -> BN(train) -> ReLU) on 8 TRN2 cores.

Sharding: data-parallel over (batch, y-strip) -> 8 shards, BN stats all-reduced.

v1 design (fp16 data paths, ~3x less HBM traffic + 4x PE rate vs the f32 baseline):

  1. Host packs each shard: occupied cells of SLOT_BLKS consecutive 128-cell
     blocks form one 128-row *slot*; root (first) points go into the per-region
     r0 tensors (fp16). Colliding extra points go into fold batches of 128
     (exf), with the root rows duplicated alongside (fi) so no device gather is
     needed. A per-batch level schedule bounds collision depth.
  2. Device folds: f = max(fi, exf levels) on DVE, then indirect-scatters f
     back into r0 *in place* (region-split r0 keeps the 4 fold chains
     independent). V tiles [128, slots, C+1] (fp16, fused ones column) load
     straight from the folded r0 -- no DRAM->DRAM comb copy.
  3. PE accumulates sig = sum_s V_s^T [V_s | 1] (fp16 in, f32 PSUM), projects
     the per-core moments q_o = w_o^T Sigma w_o, m_o = w_o . sv locally, and a
     tiny [128, 2*OCH] AllReduce(+) produces global BN stats:
     mean = m/N, var = q/N - mean^2, a = gamma/sqrt(var+eps), b = beta-mean*a.
  4. Phase C per slot-pair: the 0/1 selection matrix is rebuilt on device from
     a small row-index tensor (selrow) via a K=32 broadcast matmul + DVE
     is_equal (kills the 20MB/core sel load of the baseline); GT = V_s^T @ Sel
     densifies+transposes; conv = W^T_chunk @ GT; ACT applies relu(x*a+b) and
     the result streams out as fp16 (halves the output write).
     The first PBN pairs buffer their conv output in SBUF (fp16) so PE/DVE run
     through the AllReduce window; their ACT+store is emitted after the BN
     constants so only the scalar engine waits on the collective.
"""

import math
from dataclasses import dataclass

import numpy as np

import concourse.bass as bass
import concourse.tile as tile
from concourse import bacc, mybir
from concourse.bass_utils import run_bass_kernel_spmd

F32 = mybir.dt.float32
F16 = mybir.dt.float16
I32 = mybir.dt.int32


@dataclass(frozen=True)
class Geo:
    B: int = 2
    H: int = 400
    W: int = 400
    C: int = 128            # input channels (= partition count)
    O: int = 256            # output channels (multiple of 128)
    NSTRIP: int = 4         # y-strips per batch; B*NSTRIP = 8 cores
    SLOT_BLKS: int = 2      # 128-cell blocks packed per 128-row slot
    NB: int = 6             # fold batches per region (128 roots each)
    NREG: int = 4           # slot regions (independent r0 tensors)
    LVLS: tuple = (5, 2)    # per-batch fold depth; batches beyond get depth 1
    PBN: int = 56           # pairs whose conv output is SBUF-buffered pre-BN
    EPS: float = 1e-5

    @property
    def ystrip(self):
        return self.H // self.NSTRIP

    @property
    def cells(self):
        return self.ystrip * self.W

    @property
    def ncores(self):
        return self.B * self.NSTRIP

    @property
    def slot_cells(self):
        return 128 * self.SLOT_BLKS

    @property
    def nslot(self):
        return math.ceil(self.cells / self.slot_cells)

    @property
    def npairs(self):
        return math.ceil(self.nslot / 2)

    @property
    def lvls(self):
        return tuple(self.LVLS) + (1,) * (self.NB - len(self.LVLS))

    @property
    def npair(self):                 # (batch, level) pairs
        return sum(self.lvls)

    @property
    def ncell_total(self):
        return self.B * self.H * self.W

    @property
    def rs(self):                    # slots per region
        return math.ceil(self.nslot / self.NREG)

    @property
    def reg_bounds(self):
        out = []
        for reg in range(self.NREG):
            lo = min(reg * self.rs, self.nslot)
            hi = self.nslot if reg == self.NREG - 1 else min(
                (reg + 1) * self.rs, self.nslot)
            out.append((lo, hi))
        return out


GEO = Geo()


# --------------------------------------------------------------------------
# host-side shard prep
# --------------------------------------------------------------------------

def prep_shard(g: Geo, feats: np.ndarray, cell: np.ndarray) -> dict:
    """feats [n, C] fp16, cell [n] int in [0, g.cells)."""
    C = g.C
    order = np.argsort(cell, kind="stable")
    cell_s = cell[order]
    feats_s = feats[order]
    uniq, seg_start, inverse, counts = np.unique(
        cell_s, return_index=True, return_inverse=True, return_counts=True
    )
    rank = np.arange(len(cell_s)) - seg_start[inverse]

    # --- slot packing: cell j -> slot j // slot_cells; occupied cells of a
    # slot occupy consecutive rows (cell order) within the slot's 128 rows.
    slot_of_uniq = uniq // g.slot_cells
    occ_per_slot = np.zeros(g.nslot, np.int64)
    np.add.at(occ_per_slot, slot_of_uniq, 1)
    assert occ_per_slot.max(initial=0) <= 128, (
        f"slot overflow: {occ_per_slot.max()}"
    )
    first_of_slot = np.zeros(g.nslot, np.int64)
    first_of_slot[1:] = np.cumsum(occ_per_slot)[:-1]
    row_in_slot = np.arange(len(uniq)) - first_of_slot[slot_of_uniq]
    rowid = slot_of_uniq * 128 + row_in_slot          # global packed row

    lvls = g.lvls
    nbr = len(lvls)
    pair_base = np.cumsum((0,) + lvls[:-1])
    exi = np.zeros((128, nbr * g.NREG), np.int32)
    exf = np.zeros((128, g.npair * g.NREG, C + 1), np.float16)
    pos_in_me = np.zeros(len(uniq), np.int64)
    batch_of = np.zeros(len(uniq), np.int64)
    reg_lo = np.zeros(len(uniq), np.int64)
    shard = {}
    for reg, (lo_s, hi_s) in enumerate(g.reg_bounds):
        ns_r = hi_s - lo_s
        exi[:, reg * nbr : (reg + 1) * nbr] = (
            np.arange(128)[:, None] * (ns_r + 1) + ns_r   # per-lane dump slot
        )
        inreg = (counts > 1) & (slot_of_uniq >= lo_s) & (slot_of_uniq < hi_s)
        ord_me = np.argsort(-counts[inreg], kind="stable")
        me_uniq = np.flatnonzero(inreg)[ord_me]
        nme = len(me_uniq)
        assert nme <= 128 * nbr, f"region fold capacity exceeded: {nme}"
        bi = np.arange(nme) // 128
        pi = np.arange(nme) % 128
        assert (counts[me_uniq] - 1 <= np.asarray(lvls)[bi]).all(), (
            "collision depth exceeds fold schedule"
        )
        exi[pi, reg * nbr + bi] = (
            row_in_slot[me_uniq] * (ns_r + 1) + (slot_of_uniq[me_uniq] - lo_s)
        ).astype(np.int32)
        pos_in_me[me_uniq] = np.arange(nme)
        batch_of[me_uniq] = reg * nbr + bi
        reg_lo[me_uniq] = lo_s
    for k in range(1, int(counts.max(initial=1))):
        mk = rank == k
        if not mk.any():
            continue
        u_k = inverse[mk]
        pm = pos_in_me[u_k]
        breg = batch_of[u_k] // nbr
        bloc = batch_of[u_k] % nbr
        exf[pm % 128, breg * g.npair + pair_base[bloc] + (k - 1), :C] = feats_s[mk]

    # --- per-region r0 in partition-major layout: row = p*(ns_r+1) + s_local
    # (p = row-in-slot, s_local = slot within region, slot ns_r = dump).
    # Rows are C+1 wide with 1.0 in col C (the sigma ones-column) so the V
    # load is one contiguous (ns_r*(C+1)*2)B run per partition.
    m0 = rank == 0
    u0 = inverse[m0]
    for reg, (lo_s, hi_s) in enumerate(g.reg_bounds):
        ns_r = hi_s - lo_s
        r0r = np.zeros(((ns_r + 1) * 128, C + 1), np.float16)
        r0r[:, C] = 1.0
        sel_u = (slot_of_uniq[u0] >= lo_s) & (slot_of_uniq[u0] < hi_s)
        uu = u0[sel_u]
        rows = row_in_slot[uu] * (ns_r + 1) + (slot_of_uniq[uu] - lo_s)
        r0r[rows, :C] = feats_s[m0][sel_u]
        shard[f"r0_{reg}"] = r0r

    # --- fold init = duplicated root rows (incl. the 1.0 ones-column).
    fi = np.zeros((128, nbr * g.NREG, C + 1), np.float16)
    for reg in range(g.NREG):
        for bl in range(nbr):
            b = reg * nbr + bl
            fi[:, b, :] = shard[f"r0_{reg}"][exi[:, b]]

    # --- selrow: pair k on partition k%32, chunk k//32; value = root row of
    # the cell within its slot, or 300 (never matches iota 0..127).
    PW = 2 * g.slot_cells
    nchunk = math.ceil(g.npairs / 32)
    selrow = np.full((32, nchunk, PW), 300.0, np.float16)
    kpair = slot_of_uniq // 2
    col = (slot_of_uniq % 2) * g.slot_cells + uniq % g.slot_cells
    selrow[kpair % 32, kpair // 32, col] = row_in_slot
    shard.update({"exi": exi, "exf": exf, "fi": fi, "selrow": selrow})
    return shard


def prep_inputs(g: Geo, features, coordinates, conv_w, gamma, beta):
    feats = np.asarray(features, np.float32).astype(np.float16)
    coords = np.asarray(coordinates)
    b, y, x = coords[:, 0], coords[:, 2], coords[:, 3]
    strip = y // g.ystrip
    wt = np.ascontiguousarray(np.asarray(conv_w, np.float32).T).astype(
        np.float16)                                                 # [C, O]
    gam = np.ascontiguousarray(
        np.asarray(gamma, np.float32).reshape(g.O // 128, 128).T)   # [128, O/128]
    bet = np.ascontiguousarray(
        np.asarray(beta, np.float32).reshape(g.O // 128, 128).T)
    iota = np.arange(128, dtype=np.float32).reshape(128, 1)
    esel = np.zeros((32, 32, 128), np.float16)   # esel[r, i, :] = (r == i)
    esel[np.arange(32), np.arange(32), :] = 1.0
    in_maps = []
    for core in range(g.ncores):
        bb, st = divmod(core, g.NSTRIP)
        m = (b == bb) & (strip == st)
        cell = (y[m] - st * g.ystrip) * g.W + x[m]
        shard = prep_shard(g, feats[m], cell.astype(np.int64))
        shard.update({"wt": wt, "gamma": gam, "beta": bet, "iota": iota,
                      "esel": esel})
        in_maps.append(shard)
    return in_maps


# --------------------------------------------------------------------------
# device program
# --------------------------------------------------------------------------

def build_program(g: Geo) -> bass.Bass:
    C, O = g.C, g.O
    OCH = O // 128
    NS = g.nslot
    SC = g.slot_cells
    PW = 2 * SC
    NPAIR = g.npairs
    NCHUNK = math.ceil(NPAIR / 32)
    lvls = g.lvls
    pair_base = [0]
    for l in lvls[:-1]:
        pair_base.append(pair_base[-1] + l)
    reg_bounds = g.reg_bounds
    NBT = g.NB * g.NREG
    PBN = min(g.PBN, NPAIR)

    nc = bacc.Bacc(num_devices=g.ncores)
    r0_d = [
        nc.declare_dram_parameter(
            f"r0_{r}", [(hi - lo + 1) * 128, C + 1], F16, False)
        for r, (lo, hi) in enumerate(reg_bounds)
    ]
    exi_d = nc.declare_dram_parameter("exi", [128, NBT], I32, False)
    exf_d = nc.declare_dram_parameter(
        "exf", [128, g.npair * g.NREG, C + 1], F16, False)
    fi_d = nc.declare_dram_parameter("fi", [128, NBT, C + 1], F16, False)
    selrow_d = nc.declare_dram_parameter("selrow", [32, NCHUNK, PW], F16, False)
    wt_d = nc.declare_dram_parameter("wt", [C, O], F16, False)
    gam_d = nc.declare_dram_parameter("gamma", [128, OCH], F32, False)
    bet_d = nc.declare_dram_parameter("beta", [128, OCH], F32, False)
    iota_d = nc.declare_dram_parameter("iota", [128, 1], F32, False)
    esel_d = nc.declare_dram_parameter("esel", [32, 32, 128], F16, False)
    out_d = nc.declare_dram_parameter("out", [O, g.cells], F16, True)

    cc_in = nc.dram_tensor("cc_in", [128, 2 * OCH], F32)
    cc_out = nc.dram_tensor("cc_out", [128, 2 * OCH], F32, addr_space="Shared")

    MAX = mybir.AluOpType.max

    with tile.TileContext(nc) as tc:
        with (
            tc.tile_pool(name="vstore", bufs=1) as vstore,
            tc.tile_pool(name="singles", bufs=1) as singles,
            tc.tile_pool(name="fold", bufs=2) as fold,
            tc.tile_pool(name="selp", bufs=3) as selp,
            tc.tile_pool(name="gtp", bufs=2) as gtp,
            tc.tile_pool(name="pbn", bufs=PBN) as pbnp,
            tc.tile_pool(name="osb", bufs=4) as opool,
            tc.tile_pool(name="pstat", bufs=1, space="PSUM") as pstat,
            tc.tile_pool(name="pbrd", bufs=2, space="PSUM") as pbrd,
            tc.tile_pool(name="pgt", bufs=2, space="PSUM") as pgt,
            tc.tile_pool(name="pf", bufs=2, space="PSUM") as pf,
        ):
            # ---- small inputs
            wt16 = singles.tile([C, O], F16)
            nc.sync.dma_start(out=wt16[:], in_=wt_d[:, :])
            wt32 = singles.tile([C, O], F32)
            nc.vector.tensor_copy(out=wt32[:], in_=wt16[:])  # exact fp16->f32
            gam_sb = singles.tile([128, OCH], F32)
            nc.sync.dma_start(out=gam_sb[:], in_=gam_d[:, :])
            bet_sb = singles.tile([128, OCH], F32)
            nc.sync.dma_start(out=bet_sb[:], in_=bet_d[:, :])
            iota_sb = singles.tile([128, 1], F32)
            nc.sync.dma_start(out=iota_sb[:], in_=iota_d[:, :])
            exi_sb = singles.tile([128, NBT], I32)
            nc.scalar.dma_start(out=exi_sb[:], in_=exi_d[:, :])
            exf_sb = singles.tile([128, g.npair * g.NREG, C + 1], F16)
            nc.gpsimd.dma_start(out=exf_sb[:], in_=exf_d[:, :, :])
            fi_sb = singles.tile([128, NBT, C + 1], F16)
            nc.scalar.dma_start(out=fi_sb[:], in_=fi_d[:, :, :])
            selrow_sb = singles.tile([32, NCHUNK, PW], F16)
            nc.sync.dma_start(out=selrow_sb[:], in_=selrow_d[:, :, :])
            # esel[r, i, m] = (r == i): lhsT that broadcasts rhs row i.
            esel = singles.tile([32, 32, 128], F16)
            nc.sync.dma_start(out=esel[:], in_=esel_d[:, :, :])
            ones_f32 = singles.tile([128, 1], F32)
            nc.vector.memset(ones_f32[:], 1.0)
            eps_t = singles.tile([128, 1], F32)
            nc.vector.memset(eps_t[:], float(g.EPS))

            # ---- per-region: fold collisions into r0 in place, then load V.
            v_r = []
            for reg, (lo_s, hi_s) in enumerate(reg_bounds):
                ns_r = hi_s - lo_s
                if ns_r <= 0:
                    v_r.append(None)
                    continue
                for bl in range(g.NB):
                    b = reg * g.NB + bl
                    ft = fold.tile([128, C + 1], F16, tag="fold")
                    nc.vector.tensor_tensor(
                        out=ft[:], in0=fi_sb[:, b, :],
                        in1=exf_sb[:, reg * g.npair + pair_base[bl], :],
                        op=MAX,
                    )
                    for l in range(1, lvls[bl]):
                        nc.vector.tensor_tensor(
                            out=ft[:], in0=ft[:],
                            in1=exf_sb[:, reg * g.npair + pair_base[bl] + l, :],
                            op=MAX,
                        )
                    nc.gpsimd.indirect_dma_start(
                        out=r0_d[reg][:, :],
                        out_offset=bass.IndirectOffsetOnAxis(
                            ap=exi_sb[:, b : b + 1], axis=0
                        ),
                        in_=ft[:], in_offset=None,
                    )
                vt = vstore.tile([128, ns_r, C + 1], F16, tag=f"v{reg}")
                c3 = r0_d[reg].ap().rearrange("(p s) c -> p s c", s=ns_r + 1)
                ld_eng = [nc.sync, nc.scalar, nc.sync, nc.scalar][reg % 4]
                ld_eng.dma_start(out=vt[:, :, :], in_=c3[:, :ns_r, :])
                v_r.append(vt)

            # ---- sigma = sum_s V_s^T [V_s | 1]  -> [Sigma | sv]
            sig_ps = pstat.tile([128, C + 1], F32, space="PSUM", tag="sig")
            si = 0
            for reg, (lo_s, hi_s) in enumerate(reg_bounds):
                vt = v_r[reg]
                for sl in range(hi_s - lo_s):
                    nc.tensor.matmul(
                        out=sig_ps[:],
                        lhsT=vt[:, sl, :C],
                        rhs=vt[:, sl, :],
                        start=(si == 0), stop=(si == NS - 1),
                    )
                    si += 1
            sig_sb = singles.tile([128, C + 1], F32)
            nc.vector.tensor_copy(out=sig_sb[:], in_=sig_ps[:])

            # ---- local projected moments: q_o = w_o^T Sigma w_o, m_o = w_o.sv
            proj = pstat.tile([128, O + 2 * OCH], F32, space="PSUM", tag="proj")
            nc.tensor.matmul(
                out=proj[:, :O], lhsT=sig_sb[:, :C], rhs=wt32[:],
                start=True, stop=True,
            )
            u_sb = singles.tile([128, O], F32)
            nc.vector.tensor_tensor(
                out=u_sb[:], in0=proj[:, :O], in1=wt32[:],
                op=mybir.AluOpType.mult,
            )
            for ch in range(OCH):
                nc.tensor.matmul(
                    out=proj[:, O + ch : O + ch + 1],
                    lhsT=u_sb[:, ch * 128 : (ch + 1) * 128],
                    rhs=ones_f32[:], start=True, stop=True,
                )
                nc.tensor.matmul(
                    out=proj[:, O + OCH + ch : O + OCH + ch + 1],
                    lhsT=wt32[:, ch * 128 : (ch + 1) * 128],
                    rhs=sig_sb[:, C : C + 1], start=True, stop=True,
                )
            red_sb = singles.tile([128, 2 * OCH], F32)
            nc.vector.tensor_copy(out=red_sb[:], in_=proj[:, O : O + 2 * OCH])
            nc.sync.dma_start(out=cc_in[:, :], in_=red_sb[:])
            nc.gpsimd.collective_compute(
                "AllReduce",
                mybir.AluOpType.add,
                replica_groups=[list(range(g.ncores))],
                ins=[cc_in.ap().opt()],
                outs=[cc_out.ap().opt()],
            )

            # ---- phase C producers for the first PBN pairs (independent of
            # the collective; emitted before the BN math so the PE/DVE/GPSIMD
            # queues keep flowing while the AllReduce is in flight).
            slot_loc = []            # slot -> (region vt, local index)
            for reg, (lo_s, hi_s) in enumerate(reg_bounds):
                for sl in range(hi_s - lo_s):
                    slot_loc.append((v_r[reg], sl))

            def emit_pair_producers(k):
                base = k * PW
                w = min(PW, g.cells - base)
                s0 = 2 * k
                brd_ps = pbrd.tile([128, PW], F32, space="PSUM", tag="brd")
                nc.tensor.matmul(
                    out=brd_ps[:],
                    lhsT=esel[:, k % 32, :],
                    rhs=selrow_sb[:, k // 32, :],
                    start=True, stop=True,
                )
                sel16 = selp.tile([128, PW], F16, tag="sel")
                nc.vector.tensor_scalar(
                    sel16[:], brd_ps[:], iota_sb[:, 0:1], None,
                    mybir.AluOpType.is_equal,
                )
                gt_ps = pgt.tile([128, PW], F32, space="PSUM", tag="gt")
                w0 = min(SC, w)
                vt0, sl0 = slot_loc[s0]
                nc.tensor.matmul(
                    out=gt_ps[:, :w0],
                    lhsT=vt0[:, sl0, :C],
                    rhs=sel16[:, :w0],
                    start=True, stop=True,
                )
                if w > SC:
                    vt1, sl1 = slot_loc[s0 + 1]
                    nc.tensor.matmul(
                        out=gt_ps[:, SC:w],
                        lhsT=vt1[:, sl1, :C],
                        rhs=sel16[:, SC:w],
                        start=True, stop=True,
                    )
                gt16 = gtp.tile([128, PW], F16, tag="gt16")
                nc.vector.tensor_copy(out=gt16[:, :w], in_=gt_ps[:, :w])
                fps = []
                for ch in range(OCH):
                    fp_ps = pf.tile([128, PW], F32, space="PSUM", tag="fp")
                    nc.tensor.matmul(
                        out=fp_ps[:, :w],
                        lhsT=wt16[:, ch * 128 : (ch + 1) * 128],
                        rhs=gt16[:, :w],
                        start=True, stop=True,
                    )
                    fps.append(fp_ps)
                return w, base, fps

            def emit_pair_act(k, w, base, srcs, a_t, b_t, use_dve=False):
                for ch in range(OCH):
                    ot = opool.tile([128, PW], F16, tag=f"ot{ch}")
                    if use_dve:
                        # 2-op BN+ReLU on DVE keeps the ACT engine from
                        # becoming the serial tail of the deferred drain.
                        nc.vector.tensor_scalar(
                            ot[:, :w], srcs[ch],
                            a_t[:, ch : ch + 1], b_t[:, ch : ch + 1],
                            mybir.AluOpType.mult, mybir.AluOpType.add,
                        )
                        nc.vector.tensor_scalar(
                            ot[:, :w], ot[:, :w], 0.0, None,
                            mybir.AluOpType.max,
                        )
                    else:
                        nc.scalar.activation(
                            out=ot[:, :w], in_=srcs[ch],
                            func=mybir.ActivationFunctionType.Relu,
                            scale=a_t[:, ch : ch + 1],
                            bias=b_t[:, ch : ch + 1],
                        )
                    eng = nc.sync if k % 2 == 0 else nc.scalar
                    eng.dma_start(
                        out=out_d[ch * 128 : (ch + 1) * 128, base : base + w],
                        in_=ot[:, :w],
                    )

            deferred = []
            for k in range(PBN):
                w, base, fps = emit_pair_producers(k)
                pb = pbnp.tile([128, 2 * PW], F16, tag="pbn")
                for ch in range(OCH):
                    nc.vector.tensor_copy(
                        out=pb[:, ch * PW : ch * PW + w], in_=fps[ch][:, :w]
                    )
                deferred.append((k, w, base, pb))

            # ---- BN constants (waits on the collective)
            mom_raw = singles.tile([128, 2 * OCH], F32)
            nc.sync.dma_start(out=mom_raw[:], in_=cc_out[:, :])
            mom = singles.tile([128, 2 * OCH], F32)      # [q/N | mean]
            nc.scalar.mul(out=mom[:], in_=mom_raw[:], mul=1.0 / float(g.ncell_total))
            var_t = singles.tile([128, OCH], F32)
            nc.vector.tensor_tensor(
                out=var_t[:], in0=mom[:, OCH:], in1=mom[:, OCH:],
                op=mybir.AluOpType.mult,
            )
            nc.vector.tensor_tensor(
                out=var_t[:], in0=mom[:, :OCH], in1=var_t[:],
                op=mybir.AluOpType.subtract,
            )
            rstd = singles.tile([128, OCH], F32)
            nc.scalar.activation(
                out=rstd[:], in_=var_t[:],
                func=mybir.ActivationFunctionType.Sqrt, bias=eps_t[:],
            )
            nc.vector.reciprocal(out=rstd[:], in_=rstd[:])
            a_t = singles.tile([128, OCH], F32)
            nc.vector.tensor_tensor(
                out=a_t[:], in0=gam_sb[:], in1=rstd[:], op=mybir.AluOpType.mult
            )
            b_t = singles.tile([128, OCH], F32)
            nc.vector.tensor_tensor(
                out=b_t[:], in0=mom[:, OCH:], in1=a_t[:], op=mybir.AluOpType.mult
            )
            nc.vector.tensor_tensor(
                out=b_t[:], in0=bet_sb[:], in1=b_t[:], op=mybir.AluOpType.subtract
            )

            # ---- drain: interleave the deferred ACT+stores with the direct
            # pairs so the PE's direct-pair convs are not starved behind the
            # whole deferred drain in the ACT queue. Deferred BN alternates
            # DVE (SBUF fp16 src) / ACT engine to split the elementwise load.
            emitters = []
            for di, (k, w, base, pb) in enumerate(deferred):
                def emit_def(k=k, w=w, base=base, pb=pb, di=di):
                    srcs = [pb[:, ch * PW : ch * PW + w] for ch in range(OCH)]
                    emit_pair_act(k, w, base, srcs, a_t, b_t,
                                  use_dve=(di % 2 == 1))
                emitters.append(emit_def)
            for k in range(PBN, NPAIR):
                def emit_dir(k=k):
                    w, base, fps = emit_pair_producers(k)
                    srcs = [fps[ch][:, :w] for ch in range(OCH)]
                    emit_pair_act(k, w, base, srcs, a_t, b_t)
                emitters.append(emit_dir)
            # round-robin 2 deferred : 1 direct (56 deferred vs 23 direct)
            nd, ndir = len(deferred), NPAIR - PBN
            order = []
            i, j = 0, 0
            while i < nd or j < ndir:
                for _ in range(2):
                    if i < nd:
                        order.append(emitters[i]); i += 1
                if j < ndir:
                    order.append(emitters[nd + j]); j += 1
            for em in order:
                em()
    return nc


_PROGRAM_CACHE: dict = {}


def get_program(g: Geo) -> bass.Bass:
    if g not in _PROGRAM_CACHE:
        nc = build_program(g)
        nc.finalize()
        _PROGRAM_CACHE[g] = nc
    return _PROGRAM_CACHE[g]


def assemble_output(g: Geo, per_core: list) -> np.ndarray:
    out = np.empty((g.B, g.O, g.H, g.W), np.float32)
    for core in range(g.ncores):
        bb, st = divmod(core, g.NSTRIP)
        out[bb, :, st * g.ystrip : (st + 1) * g.ystrip, :] = (
            np.asarray(per_core[core], np.float32).reshape(g.O, g.ystrip, g.W)
        )
    return out


def kernel(features, coordinates, conv_w, gamma, beta):
    g = GEO
    in_maps = prep_inputs(g, features, coordinates, conv_w, gamma, beta)
    nc = get_program(g)
    res = run_bass_kernel_spmd(nc, in_maps, core_ids=list(range(g.ncores)))
    return assemble_output(g, [r["out"] for r in res.results])
